# revision 2
# baseline (speedup 1.0000x reference)
"""Trainium2 Bass kernel for nn_MAAttentionLayer.

Reference computation:
    combine     = concat(mashup, api)         # [N, 256], N = 16384
    combine_new = concat(mashup, new_api)     # [N, 256]
    query = combine @ q_w + q_b
    key   = combine_new @ k_w + k_b
    out   = sum((query @ key.T) / 16, axis=1) # [N]

Algebraic collapse: out[i] = query[i] . (sum_j key[j]) / 16. With
r = colsum(combine_new) and host-folded weight constants
    W  = q_w @ k_w.T, b2 = q_w @ k_b, w2 = k_w @ q_b,
this becomes
    out[i] = combine[i] . v + c
    v = (W @ r + N*b2) / 16,  c = (w2 . r + N*(q_b.k_b)) / 16
so the device only streams the big matrices once: a column-sum of
combine_new (free-axis reduce, feature dim on partitions) and a matvec
combine @ v (PE matmuls with v stationary). The 1KB partial column-sums
are combined across the 8 cores with an AllGather.

Sharding: rows of the combined node dim are split across 8 cores
(512 mashup rows + 1536 api/new_api rows per core); each core's mashup
block is read once and feeds both the column-sum and the matvec.
"""

import sys

for _p in ("/opt/trn_rl_repo", "/root/.axon_site/_ro/trn_rl_repo"):
    if _p not in sys.path:
        sys.path.insert(0, _p)

import numpy as np

import concourse.bass as bass  # noqa: E402
import concourse.mybir as mybir  # noqa: E402
import concourse.tile as tile  # noqa: E402
from concourse import bacc  # noqa: E402
from concourse.bass_utils import run_bass_kernel_spmd  # noqa: E402

NCORES = 8
P = 128
D = 256            # feature dim
KO = D // P        # feature dim chunks on partitions
M_ROWS = 4096      # mashup rows
A_ROWS = 12288     # api / new_api rows
N_TOT = M_ROWS + A_ROWS
MU_C = M_ROWS // NCORES    # 512 mashup rows per core
AP_C = A_ROWS // NCORES    # 1536 api rows per core
OUT_C = MU_C + AP_C        # 2048 output rows per core
CHUNK = 512                # matmul moving free dim (fp32 max)
MU_CHUNKS = MU_C // CHUNK  # 1
AP_CHUNKS = AP_C // CHUNK  # 3

_CACHED = {}


def _build():
    f32 = mybir.dt.float32
    nc = bacc.Bacc("TRN2", target_bir_lowering=False, debug=False,
                   num_devices=NCORES)

    mu_d = nc.dram_tensor("mu_t", [D, MU_C], f32, kind="ExternalInput")
    ap_d = nc.dram_tensor("ap_t", [D, AP_C], f32, kind="ExternalInput")
    na_d = nc.dram_tensor("na_t", [D, AP_C], f32, kind="ExternalInput")
    wt_d = nc.dram_tensor("w_t", [D, D], f32, kind="ExternalInput")
    b2_d = nc.dram_tensor("b2p", [P, KO], f32, kind="ExternalInput")
    w2_d = nc.dram_tensor("w2p", [P, KO], f32, kind="ExternalInput")
    be_d = nc.dram_tensor("beta", [1, 1], f32, kind="ExternalInput")
    out_d = nc.dram_tensor("out", [1, OUT_C], f32, kind="ExternalOutput")

    n_pieces = MU_CHUNKS + AP_CHUNKS  # reduce partials: mu pieces + na pieces
    ID = mybir.ActivationFunctionType.Identity
    inv_s = 1.0 / 16.0  # 1/sqrt(embedding_dim)

    with tile.TileContext(nc) as tc:
        with (
            tc.tile_pool(name="big", bufs=1) as big,
            tc.tile_pool(name="small", bufs=1) as small,
            tc.tile_pool(name="psum", bufs=1, space="PSUM") as psum,
            tc.tile_pool(name="dram", bufs=1, space="DRAM") as dram,
        ):
            # --- column-sum inputs first (critical path to the AllGather) ---
            mu_sb = big.tile([P, KO, MU_C], f32, tag="mu")
            nc.sync.dma_start(
                out=mu_sb[:], in_=mu_d[:].rearrange("(ko p) m -> p ko m", p=P))
            na_sb = []
            for i in range(AP_CHUNKS):
                t = big.tile([P, KO, CHUNK], f32, tag=f"na{i}", name=f"na_sb{i}")
                nc.sync.dma_start(
                    out=t[:],
                    in_=na_d[:, i * CHUNK:(i + 1) * CHUNK].rearrange(
                        "(ko p) m -> p ko m", p=P))
                na_sb.append(t)

            partials = small.tile([P, KO, n_pieces], f32)
            nc.vector.tensor_reduce(
                out=partials[:, :, 0], in_=mu_sb[:],
                axis=mybir.AxisListType.X, op=mybir.AluOpType.add)
            for i, t in enumerate(na_sb):
                nc.vector.tensor_reduce(
                    out=partials[:, :, 1 + i], in_=t[:],
                    axis=mybir.AxisListType.X, op=mybir.AluOpType.add)
            r_part = small.tile([P, KO], f32)
            nc.vector.tensor_reduce(
                out=r_part[:], in_=partials[:],
                axis=mybir.AxisListType.X, op=mybir.AluOpType.add)

            # --- AllGather the 1KB partial column-sums ---
            ag_in = dram.tile([P, KO], f32)
            ag_out = dram.tile([P * NCORES, KO], f32, addr_space="Shared")
            nc.sync.dma_start(out=ag_in[:], in_=r_part[:])
            nc.gpsimd.collective_compute(
                "AllGather", mybir.AluOpType.bypass,
                replica_groups=[list(range(NCORES))],
                ins=[ag_in[:]], outs=[ag_out[:]])
            gathered = small.tile([P, KO, NCORES], f32)
            nc.sync.dma_start(
                out=gathered[:],
                in_=ag_out[:].rearrange("(r p) ko -> p ko r", p=P))
            r = small.tile([P, KO], f32)
            nc.vector.tensor_reduce(
                out=r[:], in_=gathered[:],
                axis=mybir.AxisListType.X, op=mybir.AluOpType.add)

            # --- api rows stream in under the AllGather ---
            ap_sb = []
            for i in range(AP_CHUNKS):
                t = big.tile([P, KO, CHUNK], f32, tag=f"ap{i}", name=f"ap_sb{i}")
                nc.sync.dma_start(
                    out=t[:],
                    in_=ap_d[:, i * CHUNK:(i + 1) * CHUNK].rearrange(
                        "(ko p) m -> p ko m", p=P))
                ap_sb.append(t)

            # --- small weight tensors ---
            wt_sb = small.tile([P, KO, D], f32)
            nc.sync.dma_start(
                out=wt_sb[:], in_=wt_d[:].rearrange("(jo p) k -> p jo k", p=P))
            b2_sb = small.tile([P, KO], f32)
            nc.sync.dma_start(out=b2_sb[:], in_=b2_d[:])
            w2_sb = small.tile([P, KO], f32)
            nc.sync.dma_start(out=w2_sb[:], in_=w2_d[:])
            be_sb = small.tile([1, 1], f32)
            nc.sync.dma_start(out=be_sb[:], in_=be_d[:])

            # --- v = (W @ r)/16 + b2p   (partition-major [128, KO]) ---
            v_sb = small.tile([P, KO], f32)
            for kh in range(KO):
                pv = psum.tile([P, 1], f32, name=f"pv{kh}")
                for jo in range(KO):
                    nc.tensor.matmul(
                        out=pv[:],
                        lhsT=wt_sb[:, jo, kh * P:(kh + 1) * P],
                        rhs=r[:, jo:jo + 1],
                        start=(jo == 0), stop=(jo == KO - 1))
                nc.scalar.activation(
                    out=v_sb[:, kh:kh + 1], in_=pv[:], func=ID,
                    bias=b2_sb[:, kh:kh + 1], scale=inv_s)

            # --- c = (w2 . r)/16 + beta ---
            pc = psum.tile([1, 1], f32)
            for ko in range(KO):
                nc.tensor.matmul(
                    out=pc[:], lhsT=r[:, ko:ko + 1], rhs=w2_sb[:, ko:ko + 1],
                    start=(ko == 0), stop=(ko == KO - 1))
            c_sb = small.tile([1, 1], f32)
            nc.scalar.activation(
                out=c_sb[:], in_=pc[:], func=ID, bias=be_sb[:, 0:1],
                scale=inv_s)

            # --- big matvec: out = X . v + c, X pieces stream through PE ---
            out_sb = small.tile([1, OUT_C], f32)
            pieces = [mu_sb] + ap_sb
            for n, piece in enumerate(pieces):
                po = psum.tile([1, CHUNK], f32, name=f"po{n}")
                for ko in range(KO):
                    nc.tensor.matmul(
                        out=po[:], lhsT=v_sb[:, ko:ko + 1],
                        rhs=piece[:, ko, :],
                        start=(ko == 0), stop=(ko == KO - 1))
                nc.scalar.activation(
                    out=out_sb[:, n * CHUNK:(n + 1) * CHUNK], in_=po[:],
                    func=ID, bias=c_sb[:, 0:1], scale=1.0)

            nc.sync.dma_start(out=out_d[:], in_=out_sb[:])

    nc.compile()
    return nc


def _get_nc():
    if "nc" not in _CACHED:
        _CACHED["nc"] = _build()
    return _CACHED["nc"]


def kernel(mashup_embeddings, api_embeddings, new_api_embeddings,
           q_w, q_b, k_w, k_b, embedding_dim):
    mu = np.asarray(mashup_embeddings, dtype=np.float32)
    ap = np.asarray(api_embeddings, dtype=np.float32)
    na = np.asarray(new_api_embeddings, dtype=np.float32)
    qw = np.asarray(q_w, dtype=np.float64)
    qb = np.asarray(q_b, dtype=np.float64)
    kw = np.asarray(k_w, dtype=np.float64)
    kb = np.asarray(k_b, dtype=np.float64)
    n_tot = mu.shape[0] + ap.shape[0]
    inv_s = 1.0 / np.sqrt(float(np.asarray(embedding_dim)))
    assert abs(inv_s - 1.0 / 16.0) < 1e-12, "kernel hardcodes 1/sqrt(256)"

    # Host-folded weight constants (O(d^3), independent of the node count).
    wt = np.ascontiguousarray((kw @ qw.T).astype(np.float32))      # [D, D]
    b2 = (n_tot * inv_s) * (qw @ kb)                               # [D]
    w2 = kw @ qb                                                   # [D]
    beta = np.float32(n_tot * inv_s * float(qb @ kb))
    b2p = np.ascontiguousarray(b2.reshape(KO, P).T.astype(np.float32))
    w2p = np.ascontiguousarray(w2.reshape(KO, P).T.astype(np.float32))
    be = np.full((1, 1), beta, dtype=np.float32)

    in_maps = []
    for c in range(NCORES):
        mu_c = mu[c * MU_C:(c + 1) * MU_C]
        ap_c = ap[c * AP_C:(c + 1) * AP_C]
        na_c = na[c * AP_C:(c + 1) * AP_C]
        in_maps.append({
            "mu_t": np.ascontiguousarray(mu_c.T),
            "ap_t": np.ascontiguousarray(ap_c.T),
            "na_t": np.ascontiguousarray(na_c.T),
            "w_t": wt,
            "b2p": b2p,
            "w2p": w2p,
            "beta": be,
        })

    nc = _get_nc()
    _CACHED["in_maps"] = in_maps
    res = run_bass_kernel_spmd(nc, in_maps, core_ids=list(range(NCORES)))

    out = np.empty(n_tot, dtype=np.float32)
    for c in range(NCORES):
        oc = res.results[c]["out"].reshape(OUT_C)
        out[c * MU_C:(c + 1) * MU_C] = oc[:MU_C]
        out[M_ROWS + c * AP_C:M_ROWS + (c + 1) * AP_C] = oc[MU_C:]
    return out


# revision 3
# speedup vs baseline: 1.8485x; 1.8485x over previous
"""Trainium2 Bass kernel for nn_MAAttentionLayer.

Reference computation (N = 16384, d = 256):
    combine     = concat(mashup, api)         # [N, d]
    combine_new = concat(mashup, new_api)     # [N, d]
    query = combine @ q_w + q_b
    key   = combine_new @ k_w + k_b
    out   = sum((query @ key.T) / sqrt(d), axis=1)     # [N]

Algebraic collapse: out[i] = query[i] . (sum_j key[j]) / sqrt(d).
With r = colsum(combine_new) and host-folded weight constants
    W = q_w @ k_w.T,  b2 = q_w @ k_b,  w2 = k_w @ q_b,
this becomes
    out[i] = combine[i] . v + c
    v = (W @ r + N*b2)/sqrt(d),  c = (w2 . r + N*(q_b.k_b))/sqrt(d)
so the device only has to stream each big matrix once.

Two SPMD launches over 8 cores (in-kernel collectives cost 30-50us on
this stack, launch overhead is ~15us, so two launches win):
  L1: each core reduces its 2048-row shard of combine_new^T along the
      free axis -> partial colsum [128, 2] (feature dim on partitions).
      Host sums the 8 partials and folds the weights into v, c.
  L2: each core computes its 2048 rows of combine . v + c with
      v-stationary PE matmuls over host-transposed shards.
A DVE-based L2 variant (natural layout, tensor_tensor + reduce) is kept
for A/B testing; PE is the default.
"""

import sys

for _p in ("/opt/trn_rl_repo", "/root/.axon_site/_ro/trn_rl_repo"):
    if _p not in sys.path:
        sys.path.insert(0, _p)

import numpy as np

import concourse.bass as bass  # noqa: E402
import concourse.mybir as mybir  # noqa: E402
import concourse.tile as tile  # noqa: E402
from concourse import bacc  # noqa: E402
from concourse.bass_utils import run_bass_kernel_spmd  # noqa: E402

NCORES = 8
P = 128
D = 256            # feature dim
KO = D // P        # feature-dim chunks on partitions
M_ROWS = 4096      # mashup rows
A_ROWS = 12288     # api / new_api rows
N_TOT = M_ROWS + A_ROWS
MU_C = M_ROWS // NCORES    # 512 mashup rows per core
AP_C = A_ROWS // NCORES    # 1536 api rows per core
OUT_C = MU_C + AP_C        # 2048 rows per core
CHUNK = 512                # columns per pipelined piece
N_PIECES = OUT_C // CHUNK  # 4

L2_VARIANT = "pe"          # "pe" | "dve"

_CACHED = {}
_f32 = mybir.dt.float32


def _build_l1():
    """Partial column-sum of the core's combine_new^T shard."""
    nc = bacc.Bacc("TRN2", target_bir_lowering=False, debug=False,
                   num_devices=NCORES, name="maatt_l1")
    cn_d = nc.dram_tensor("cn_t", [D, OUT_C], _f32, kind="ExternalInput")
    r_d = nc.dram_tensor("r_part", [P, KO], _f32, kind="ExternalOutput")

    with tile.TileContext(nc) as tc:
        with tc.tile_pool(name="sb", bufs=1) as sb:
            partials = sb.tile([P, KO, N_PIECES], _f32)
            for i in range(N_PIECES):
                t = sb.tile([P, KO, CHUNK], _f32, tag=f"cn{i}", name=f"cn{i}")
                nc.sync.dma_start(
                    out=t[:],
                    in_=cn_d[:, i * CHUNK:(i + 1) * CHUNK].rearrange(
                        "(ko p) m -> p ko m", p=P))
                nc.vector.tensor_reduce(
                    out=partials[:, :, i], in_=t[:],
                    axis=mybir.AxisListType.X, op=mybir.AluOpType.add)
            r_sb = sb.tile([P, KO], _f32)
            nc.vector.tensor_reduce(
                out=r_sb[:], in_=partials[:],
                axis=mybir.AxisListType.X, op=mybir.AluOpType.add)
            nc.sync.dma_start(out=r_d[:], in_=r_sb[:])
    nc.compile()
    return nc


def _build_l2_pe():
    """out = X . v + c with v-stationary PE matmuls over X^T pieces."""
    nc = bacc.Bacc("TRN2", target_bir_lowering=False, debug=False,
                   num_devices=NCORES, name="maatt_l2")
    x_d = nc.dram_tensor("x_t", [D, OUT_C], _f32, kind="ExternalInput")
    v_d = nc.dram_tensor("v_p", [P, KO], _f32, kind="ExternalInput")
    c_d = nc.dram_tensor("c_p", [1, 1], _f32, kind="ExternalInput")
    o_d = nc.dram_tensor("o", [1, OUT_C], _f32, kind="ExternalOutput")
    ID = mybir.ActivationFunctionType.Identity

    with tile.TileContext(nc) as tc:
        with (
            tc.tile_pool(name="sb", bufs=1) as sb,
            tc.tile_pool(name="ps", bufs=1, space="PSUM") as ps,
        ):
            v_sb = sb.tile([P, KO], _f32)
            nc.sync.dma_start(out=v_sb[:], in_=v_d[:])
            c_sb = sb.tile([1, 1], _f32)
            nc.sync.dma_start(out=c_sb[:], in_=c_d[:])
            o_sb = sb.tile([1, OUT_C], _f32)
            for i in range(N_PIECES):
                t = sb.tile([P, KO, CHUNK], _f32, tag=f"x{i}", name=f"x{i}")
                nc.sync.dma_start(
                    out=t[:],
                    in_=x_d[:, i * CHUNK:(i + 1) * CHUNK].rearrange(
                        "(ko p) m -> p ko m", p=P))
                po = ps.tile([1, CHUNK], _f32, name=f"po{i}")
                for ko in range(KO):
                    nc.tensor.matmul(
                        out=po[:], lhsT=v_sb[:, ko:ko + 1], rhs=t[:, ko, :],
                        start=(ko == 0), stop=(ko == KO - 1))
                nc.scalar.activation(
                    out=o_sb[:, i * CHUNK:(i + 1) * CHUNK], in_=po[:],
                    func=ID, bias=c_sb[:, 0:1], scale=1.0)
            nc.sync.dma_start(out=o_d[:], in_=o_sb[:])
    nc.compile()
    return nc


def _build_l2_dve():
    """out = X . v + c with natural-layout DVE mult+reduce."""
    nc = bacc.Bacc("TRN2", target_bir_lowering=False, debug=False,
                   num_devices=NCORES, name="maatt_l2d")
    TPP = OUT_C // P          # 16 row-tiles of 128
    TPC = TPP // N_PIECES     # 4 row-tiles per piece
    x_d = nc.dram_tensor("x_n", [OUT_C, D], _f32, kind="ExternalInput")
    v_d = nc.dram_tensor("v_rep", [P, D], _f32, kind="ExternalInput")
    c_d = nc.dram_tensor("c_rep", [P, 1], _f32, kind="ExternalInput")
    o_d = nc.dram_tensor("o", [P, TPP], _f32, kind="ExternalOutput")

    with tile.TileContext(nc) as tc:
        with tc.tile_pool(name="sb", bufs=1) as sb:
            v_sb = sb.tile([P, D], _f32)
            nc.sync.dma_start(out=v_sb[:], in_=v_d[:])
            c_sb = sb.tile([P, 1], _f32)
            nc.sync.dma_start(out=c_sb[:], in_=c_d[:])
            red = sb.tile([P, TPP], _f32)
            for i in range(N_PIECES):
                t = sb.tile([P, TPC, D], _f32, tag=f"x{i}", name=f"x{i}")
                nc.sync.dma_start(
                    out=t[:],
                    in_=x_d[i * TPC * P:(i + 1) * TPC * P, :].rearrange(
                        "(t p) k -> p t k", p=P))
                nc.vector.tensor_tensor(
                    t[:], t[:],
                    v_sb[:, None, :].to_broadcast((P, TPC, D)),
                    mybir.AluOpType.mult)
                nc.vector.tensor_reduce(
                    out=red[:, i * TPC:(i + 1) * TPC], in_=t[:],
                    axis=mybir.AxisListType.X, op=mybir.AluOpType.add)
            o_sb = sb.tile([P, TPP], _f32)
            nc.vector.tensor_scalar(
                o_sb[:], red[:], c_sb[:, 0:1], None, mybir.AluOpType.add)
            nc.sync.dma_start(out=o_d[:], in_=o_sb[:])
    nc.compile()
    return nc


def _get(name, builder):
    if name not in _CACHED:
        _CACHED[name] = builder()
    return _CACHED[name]


def _host_fold(q_w, q_b, k_w, k_b, n_tot, inv_s, r):
    """v = (W@r + N*b2)*inv_s, c = (w2.r + N*(q_b.k_b))*inv_s in f64."""
    v = inv_s * (q_w @ (k_w.T @ r) + n_tot * (q_w @ k_b))
    c = inv_s * (float(k_w @ q_b @ r) + n_tot * float(q_b @ k_b))
    return v, c


def kernel(mashup_embeddings, api_embeddings, new_api_embeddings,
           q_w, q_b, k_w, k_b, embedding_dim):
    mu = np.asarray(mashup_embeddings, dtype=np.float32)
    ap = np.asarray(api_embeddings, dtype=np.float32)
    na = np.asarray(new_api_embeddings, dtype=np.float32)
    qw = np.asarray(q_w, dtype=np.float64)
    qb = np.asarray(q_b, dtype=np.float64)
    kw = np.asarray(k_w, dtype=np.float64)
    kb = np.asarray(k_b, dtype=np.float64)
    n_tot = mu.shape[0] + ap.shape[0]
    inv_s = 1.0 / np.sqrt(float(np.asarray(embedding_dim)))

    core_ids = list(range(NCORES))

    # ---- L1: partial column-sums of combine_new ----
    nc1 = _get("l1", _build_l1)
    in1 = []
    for c in range(NCORES):
        shard = np.concatenate(
            [mu[c * MU_C:(c + 1) * MU_C], na[c * AP_C:(c + 1) * AP_C]], axis=0)
        in1.append({"cn_t": np.ascontiguousarray(shard.T)})
    _CACHED["in1"] = in1
    res1 = run_bass_kernel_spmd(nc1, in1, core_ids)

    rp = np.zeros((P, KO), dtype=np.float64)
    for c in range(NCORES):
        rp += res1.results[c]["r_part"].astype(np.float64)
    r = rp.T.reshape(D)          # r[k], k = ko*128 + p

    v, cc = _host_fold(qw, qb, kw, kb, n_tot, inv_s, r)

    # ---- L2: out = combine . v + c ----
    out = np.empty(n_tot, dtype=np.float32)
    if L2_VARIANT == "pe":
        nc2 = _get("l2pe", _build_l2_pe)
        v_p = np.ascontiguousarray(
            v.reshape(KO, P).T.astype(np.float32))          # [128, KO]
        c_p = np.full((1, 1), cc, dtype=np.float32)
        in2 = []
        for c in range(NCORES):
            shard = np.concatenate(
                [mu[c * MU_C:(c + 1) * MU_C], ap[c * AP_C:(c + 1) * AP_C]],
                axis=0)
            in2.append({"x_t": np.ascontiguousarray(shard.T),
                        "v_p": v_p, "c_p": c_p})
        _CACHED["in2"] = in2
        res2 = run_bass_kernel_spmd(nc2, in2, core_ids)
        for c in range(NCORES):
            oc = res2.results[c]["o"].reshape(OUT_C)
            out[c * MU_C:(c + 1) * MU_C] = oc[:MU_C]
            out[M_ROWS + c * AP_C:M_ROWS + (c + 1) * AP_C] = oc[MU_C:]
    else:
        nc2 = _get("l2dve", _build_l2_dve)
        v_rep = np.ascontiguousarray(
            np.broadcast_to(v.astype(np.float32), (P, D)))
        c_rep = np.full((P, 1), cc, dtype=np.float32)
        in2 = []
        for c in range(NCORES):
            shard = np.concatenate(
                [mu[c * MU_C:(c + 1) * MU_C], ap[c * AP_C:(c + 1) * AP_C]],
                axis=0)
            in2.append({"x_n": np.ascontiguousarray(shard),
                        "v_rep": v_rep, "c_rep": c_rep})
        _CACHED["in2"] = in2
        res2 = run_bass_kernel_spmd(nc2, in2, core_ids)
        for c in range(NCORES):
            oc = res2.results[c]["o"]            # [128, 16], row = t*128+p
            oc = oc.T.reshape(OUT_C)
            out[c * MU_C:(c + 1) * MU_C] = oc[:MU_C]
            out[M_ROWS + c * AP_C:M_ROWS + (c + 1) * AP_C] = oc[MU_C:]
    return out


# revision 4
# speedup vs baseline: 1.8934x; 1.0243x over previous
"""Trainium2 Bass kernel for nn_MAAttentionLayer.

Reference computation (N = 16384, d = 256):
    combine     = concat(mashup, api)         # [N, d]
    combine_new = concat(mashup, new_api)     # [N, d]
    query = combine @ q_w + q_b
    key   = combine_new @ k_w + k_b
    out   = sum((query @ key.T) / sqrt(d), axis=1)     # [N]

Algebraic collapse: out[i] = query[i] . (sum_j key[j]) / sqrt(d).
With r = colsum(combine_new) and host-folded weight constants
    W = q_w @ k_w.T,  b2 = q_w @ k_b,  w2 = k_w @ q_b,
this becomes
    out[i] = combine[i] . v + c
    v = (W @ r + N*b2)/sqrt(d),  c = (w2 . r + N*(q_b.k_b))/sqrt(d)
so the device only has to stream each big matrix once.

Two SPMD launches over 8 cores (in-kernel collectives cost 30-50us on
this stack, launch overhead is ~15us, so two launches win):
  L1: each core reduces its 2048-row shard of combine_new^T along the
      free axis -> partial colsum [128, 2] (feature dim on partitions).
      Host sums the 8 partials and folds the weights into v, c.
  L2: each core computes its 2048 rows of combine . v + c with
      v-stationary PE matmuls over host-transposed shards.
A DVE-based L2 variant (natural layout, tensor_tensor + reduce) is kept
for A/B testing; PE is the default.
"""

import sys

for _p in ("/opt/trn_rl_repo", "/root/.axon_site/_ro/trn_rl_repo"):
    if _p not in sys.path:
        sys.path.insert(0, _p)

import numpy as np

import concourse.bass as bass  # noqa: E402
import concourse.mybir as mybir  # noqa: E402
import concourse.tile as tile  # noqa: E402
from concourse import bacc  # noqa: E402
from concourse.bass_utils import run_bass_kernel_spmd  # noqa: E402

NCORES = 8
P = 128
D = 256            # feature dim
KO = D // P        # feature-dim chunks on partitions
M_ROWS = 4096      # mashup rows
A_ROWS = 12288     # api / new_api rows
N_TOT = M_ROWS + A_ROWS
MU_C = M_ROWS // NCORES    # 512 mashup rows per core
AP_C = A_ROWS // NCORES    # 1536 api rows per core
OUT_C = MU_C + AP_C        # 2048 rows per core
CHUNK = 512                # columns per pipelined piece
N_PIECES = OUT_C // CHUNK  # 4

L2_VARIANT = "dve"          # "pe" | "dve"

_CACHED = {}
_f32 = mybir.dt.float32


def _build_l1():
    """Partial column-sum of the core's combine_new^T shard."""
    nc = bacc.Bacc("TRN2", target_bir_lowering=False, debug=False,
                   num_devices=NCORES, name="maatt_l1")
    cn_d = nc.dram_tensor("cn_t", [D, OUT_C], _f32, kind="ExternalInput")
    r_d = nc.dram_tensor("r_part", [P, KO], _f32, kind="ExternalOutput")

    with tile.TileContext(nc) as tc:
        with tc.tile_pool(name="sb", bufs=1) as sb:
            partials = sb.tile([P, KO, N_PIECES], _f32)
            for i in range(N_PIECES):
                t = sb.tile([P, KO, CHUNK], _f32, tag=f"cn{i}", name=f"cn{i}")
                nc.sync.dma_start(
                    out=t[:],
                    in_=cn_d[:, i * CHUNK:(i + 1) * CHUNK].rearrange(
                        "(ko p) m -> p ko m", p=P))
                nc.vector.tensor_reduce(
                    out=partials[:, :, i], in_=t[:],
                    axis=mybir.AxisListType.X, op=mybir.AluOpType.add)
            r_sb = sb.tile([P, KO], _f32)
            nc.vector.tensor_reduce(
                out=r_sb[:], in_=partials[:],
                axis=mybir.AxisListType.X, op=mybir.AluOpType.add)
            nc.sync.dma_start(out=r_d[:], in_=r_sb[:])
    nc.compile()
    return nc


def _build_l2_pe():
    """out = X . v + c with v-stationary PE matmuls over X^T pieces."""
    nc = bacc.Bacc("TRN2", target_bir_lowering=False, debug=False,
                   num_devices=NCORES, name="maatt_l2")
    x_d = nc.dram_tensor("x_t", [D, OUT_C], _f32, kind="ExternalInput")
    v_d = nc.dram_tensor("v_p", [P, KO], _f32, kind="ExternalInput")
    c_d = nc.dram_tensor("c_p", [1, 1], _f32, kind="ExternalInput")
    o_d = nc.dram_tensor("o", [1, OUT_C], _f32, kind="ExternalOutput")
    ID = mybir.ActivationFunctionType.Identity

    with tile.TileContext(nc) as tc:
        with (
            tc.tile_pool(name="sb", bufs=1) as sb,
            tc.tile_pool(name="ps", bufs=1, space="PSUM") as ps,
        ):
            v_sb = sb.tile([P, KO], _f32)
            nc.sync.dma_start(out=v_sb[:], in_=v_d[:])
            c_sb = sb.tile([1, 1], _f32)
            nc.sync.dma_start(out=c_sb[:], in_=c_d[:])
            o_sb = sb.tile([1, OUT_C], _f32)
            for i in range(N_PIECES):
                t = sb.tile([P, KO, CHUNK], _f32, tag=f"x{i}", name=f"x{i}")
                nc.sync.dma_start(
                    out=t[:],
                    in_=x_d[:, i * CHUNK:(i + 1) * CHUNK].rearrange(
                        "(ko p) m -> p ko m", p=P))
                po = ps.tile([1, CHUNK], _f32, name=f"po{i}")
                for ko in range(KO):
                    nc.tensor.matmul(
                        out=po[:], lhsT=v_sb[:, ko:ko + 1], rhs=t[:, ko, :],
                        start=(ko == 0), stop=(ko == KO - 1))
                nc.scalar.activation(
                    out=o_sb[:, i * CHUNK:(i + 1) * CHUNK], in_=po[:],
                    func=ID, bias=c_sb[:, 0:1], scale=1.0)
            nc.sync.dma_start(out=o_d[:], in_=o_sb[:])
    nc.compile()
    return nc


def _build_l2_dve():
    """out = X . v + c with natural-layout DVE mult+reduce."""
    nc = bacc.Bacc("TRN2", target_bir_lowering=False, debug=False,
                   num_devices=NCORES, name="maatt_l2d")
    TPP = OUT_C // P          # 16 row-tiles of 128
    TPC = TPP // N_PIECES     # 4 row-tiles per piece
    x_d = nc.dram_tensor("x_n", [OUT_C, D], _f32, kind="ExternalInput")
    v_d = nc.dram_tensor("v_rep", [P, D], _f32, kind="ExternalInput")
    c_d = nc.dram_tensor("c_rep", [P, 1], _f32, kind="ExternalInput")
    o_d = nc.dram_tensor("o", [P, TPP], _f32, kind="ExternalOutput")

    with tile.TileContext(nc) as tc:
        with tc.tile_pool(name="sb", bufs=1) as sb:
            v_sb = sb.tile([P, D], _f32)
            nc.sync.dma_start(out=v_sb[:], in_=v_d[:])
            c_sb = sb.tile([P, 1], _f32)
            nc.sync.dma_start(out=c_sb[:], in_=c_d[:])
            red = sb.tile([P, TPP], _f32)
            for i in range(N_PIECES):
                t = sb.tile([P, TPC, D], _f32, tag=f"x{i}", name=f"x{i}")
                nc.sync.dma_start(
                    out=t[:],
                    in_=x_d[i * TPC * P:(i + 1) * TPC * P, :].rearrange(
                        "(t p) k -> p t k", p=P))
                nc.vector.tensor_tensor(
                    t[:], t[:],
                    v_sb[:, None, :].to_broadcast((P, TPC, D)),
                    mybir.AluOpType.mult)
                nc.vector.tensor_reduce(
                    out=red[:, i * TPC:(i + 1) * TPC], in_=t[:],
                    axis=mybir.AxisListType.X, op=mybir.AluOpType.add)
            o_sb = sb.tile([P, TPP], _f32)
            nc.vector.tensor_scalar(
                o_sb[:], red[:], c_sb[:, 0:1], None, mybir.AluOpType.add)
            nc.sync.dma_start(out=o_d[:], in_=o_sb[:])
    nc.compile()
    return nc


def _get(name, builder):
    if name not in _CACHED:
        _CACHED[name] = builder()
    return _CACHED[name]


def _host_fold(q_w, q_b, k_w, k_b, n_tot, inv_s, r):
    """v = (W@r + N*b2)*inv_s, c = (w2.r + N*(q_b.k_b))*inv_s in f64."""
    v = inv_s * (q_w @ (k_w.T @ r) + n_tot * (q_w @ k_b))
    c = inv_s * (float(k_w @ q_b @ r) + n_tot * float(q_b @ k_b))
    return v, c


def kernel(mashup_embeddings, api_embeddings, new_api_embeddings,
           q_w, q_b, k_w, k_b, embedding_dim):
    mu = np.asarray(mashup_embeddings, dtype=np.float32)
    ap = np.asarray(api_embeddings, dtype=np.float32)
    na = np.asarray(new_api_embeddings, dtype=np.float32)
    qw = np.asarray(q_w, dtype=np.float64)
    qb = np.asarray(q_b, dtype=np.float64)
    kw = np.asarray(k_w, dtype=np.float64)
    kb = np.asarray(k_b, dtype=np.float64)
    n_tot = mu.shape[0] + ap.shape[0]
    inv_s = 1.0 / np.sqrt(float(np.asarray(embedding_dim)))

    core_ids = list(range(NCORES))

    # ---- L1: partial column-sums of combine_new ----
    nc1 = _get("l1", _build_l1)
    in1 = []
    for c in range(NCORES):
        shard = np.concatenate(
            [mu[c * MU_C:(c + 1) * MU_C], na[c * AP_C:(c + 1) * AP_C]], axis=0)
        in1.append({"cn_t": np.ascontiguousarray(shard.T)})
    _CACHED["in1"] = in1
    res1 = run_bass_kernel_spmd(nc1, in1, core_ids)

    rp = np.zeros((P, KO), dtype=np.float64)
    for c in range(NCORES):
        rp += res1.results[c]["r_part"].astype(np.float64)
    r = rp.T.reshape(D)          # r[k], k = ko*128 + p

    v, cc = _host_fold(qw, qb, kw, kb, n_tot, inv_s, r)

    # ---- L2: out = combine . v + c ----
    out = np.empty(n_tot, dtype=np.float32)
    if L2_VARIANT == "pe":
        nc2 = _get("l2pe", _build_l2_pe)
        v_p = np.ascontiguousarray(
            v.reshape(KO, P).T.astype(np.float32))          # [128, KO]
        c_p = np.full((1, 1), cc, dtype=np.float32)
        in2 = []
        for c in range(NCORES):
            shard = np.concatenate(
                [mu[c * MU_C:(c + 1) * MU_C], ap[c * AP_C:(c + 1) * AP_C]],
                axis=0)
            in2.append({"x_t": np.ascontiguousarray(shard.T),
                        "v_p": v_p, "c_p": c_p})
        _CACHED["in2"] = in2
        res2 = run_bass_kernel_spmd(nc2, in2, core_ids)
        for c in range(NCORES):
            oc = res2.results[c]["o"].reshape(OUT_C)
            out[c * MU_C:(c + 1) * MU_C] = oc[:MU_C]
            out[M_ROWS + c * AP_C:M_ROWS + (c + 1) * AP_C] = oc[MU_C:]
    else:
        nc2 = _get("l2dve", _build_l2_dve)
        v_rep = np.ascontiguousarray(
            np.broadcast_to(v.astype(np.float32), (P, D)))
        c_rep = np.full((P, 1), cc, dtype=np.float32)
        in2 = []
        for c in range(NCORES):
            shard = np.concatenate(
                [mu[c * MU_C:(c + 1) * MU_C], ap[c * AP_C:(c + 1) * AP_C]],
                axis=0)
            in2.append({"x_n": np.ascontiguousarray(shard),
                        "v_rep": v_rep, "c_rep": c_rep})
        _CACHED["in2"] = in2
        res2 = run_bass_kernel_spmd(nc2, in2, core_ids)
        for c in range(NCORES):
            oc = res2.results[c]["o"]            # [128, 16], row = t*128+p
            oc = oc.T.reshape(OUT_C)
            out[c * MU_C:(c + 1) * MU_C] = oc[:MU_C]
            out[M_ROWS + c * AP_C:M_ROWS + (c + 1) * AP_C] = oc[MU_C:]
    return out
